# revision 66
# baseline (speedup 1.0000x reference)
"""LowRankAttention Trainium2 kernel (8-core SPMD), v2.

Sharding: core c handles batch b = c//2 and query-half sh = c%2.  The host
rolls the sequence axis of x[b] by -1024*sh so every core's program is
identical: query rows are rolled-rows [0, 1024); keys/values span the full
(rolled) 2048 rows.  Softmax/AV are invariant to the key permutation.

Algebra (per head h; parameter folding on host):
  tT       = qkv_u^T @ x^T                      [32, 2048]   (aug row 32 = ones)
  k_lowT_h = Wk_h^T @ tT_aug                    (Wk_h = Vk_h U_h; bias via ones row)
  q_lowT_h = Wq_h^T @ tT_aug * scale
  scoresT  = k_lowT^T @ q_lowT  on PE (rank-32, bf16)
  ex       = exp(scoresT) on ACT (bf16) -- the wall: ~33M exps/core
  uz_h     = [Wg; ones]^T-projected AV: V'_h = v_low_h @ (va_h @ outu_h), so
             uz rows 0..31 are already g-space; row 32 = Z (ones col in V_sb)
  g        = sum_h uz_h[0:32] * (1/Z_h)         (recip on DVE, partition bcast
             on the idle GPSIMD engine, PSUM*SBUF mul on DVE, identity-lhsT
             accumulation matmul on PE)
  y        = [g; ones]^T @ [out_v; out_b]

Loop order: query-half (sbq) outer, heads inner.  g accumulates per head in
PSUM, so the output projection of half 0 overlaps half 1's attention and the
kernel tail is only the last head's finalize + 4 y-chunks.

Engine budget (cost model): ACT 256 exps x (1024+222)/1.2 = 266us is the
wall; PE ~262us (scores 109 + AV 109 + kq 20 + transposes/tT/v 17 + g/y 7);
DVE ~135us; Pool ~26us.  PSUM: sc 2x2 + cty 1 + uz 2 + g 1 = 8 banks.

dtypes: x/xT/tT/projection params float32r (transposes 1.5 cyc/row, matmuls
1 cyc/row at free>=256); K/Q/V/ex bf16; fp32r producers (DVE copies) emit
float32r-typed outputs (walrus rule).  K/Q live unstacked at partitions
0-31 ([32, H, S] bf16), so projection results copy PSUM->SBUF on DVE with
no partition move and no SBUF->SBUF stacking DMAs.
"""

import os

import numpy as np

import concourse.bass as bass
import concourse.mybir as mybir
import concourse.tile as tile
from concourse import bacc
from concourse.bass_utils import run_bass_kernel_spmd
from concourse.masks import make_identity

F32 = mybir.dt.float32
F32R = mybir.dt.float32r
BF16 = mybir.dt.bfloat16
EXP = mybir.ActivationFunctionType.Exp

B, S, D = 4, 2048, 1024
H, HD, R = 16, 64, 32
SHALF = S // 2          # query rows per core
NC = 8


def build_program():
    # Bacc (not raw Bass): its compile() splits multi-semaphore waits into
    # EventSemaphore instructions and moves matmul waits onto LDWEIGHTS --
    # TPB instructions have a single wait slot.
    nc = bacc.Bacc("TRN2", target_bir_lowering=False, debug=False)

    xb = nc.dram_tensor("xb", [S, D], BF16, kind="ExternalInput").ap()
    wq = nc.dram_tensor("wq", [R + 1, H * R], F32R, kind="ExternalInput").ap()
    wk = nc.dram_tensor("wk", [R + 1, H * R], F32R, kind="ExternalInput").ap()
    wv = nc.dram_tensor("wv", [R + 1, H * R], F32R, kind="ExternalInput").ap()
    qkvu = nc.dram_tensor("qkvu", [D, R], BF16, kind="ExternalInput").ap()
    outv = nc.dram_tensor("outv", [R + 1, D], F32R, kind="ExternalInput").ap()
    ones_d = nc.dram_tensor("ones2048", [1, S], F32R, kind="ExternalInput").ap()
    y = nc.dram_tensor("y", [SHALF, D], F32, kind="ExternalOutput").ap()

    with tile.TileContext(nc) as tc:
        with tc.tile_pool(name="persist", bufs=1) as persist:
            # ---- parameters into SBUF ----
            identf = persist.tile([128, 128], F32)
            make_identity(nc, identf)
            # f32r copy: walrus requires fp32r-matmul operands to come from
            # fp32r-typed producers; DVE copy is one.
            ident = persist.tile([128, 128], F32R)
            nc.vector.tensor_copy(ident, identf)
            # The DMA device is exclusive per transfer in the cost model and
            # the 8MB x input takes ~23us on it, so param DMA emission is
            # interleaved with the x chunks below: only what the first
            # x-blocks' consumers need goes first.
            identb = persist.tile([128, 128], BF16)
            make_identity(nc, identb)
            qkvu_sb = persist.tile([128, 8, R], BF16)
            nc.sync.dma_start(out=qkvu_sb, in_=qkvu.rearrange("(a p) r -> p a r", p=128))
            wq_sb = persist.tile([R + 1, H * R], F32R)
            wk_sb = persist.tile([R + 1, H * R], F32R)
            wv_sb = persist.tile([R + 1, H * R], F32R)
            outv_sb = persist.tile([R + 1, D], F32R)

            zeros_col = persist.tile([128, 1], F32)
            nc.vector.memset(zeros_col, 0.0)
            # ACT warm-up: observe DVE's memset tick before the first real exp
            scratch_sb = persist.tile([128, 1], F32)
            nc.scalar.activation(scratch_sb, zeros_col, EXP, bias=zeros_col)

            # ---- persistent activations ----
            # tT in 4 separate 512-col tiles so the early k/q pieces depend
            # only on the x blocks actually needed.
            tTb = [persist.tile([R + 1, 512], F32R, name=f"tTb{j}")
                   for j in range(4)]
            K_sb = persist.tile([R, H, S], BF16)       # [r, h, t]
            Q_sb = persist.tile([R, H, SHALF], BF16)   # [r, h, s]
            V_sb = persist.tile([128, 16, H, R + 1], BF16)  # [tp, tc, h, g|ones]
            # ones column via the idle GPSIMD engine (a DMA would cost 14us
            # of per-element descriptors on the strided destination)
            nc.gpsimd.memset(V_sb[:, :, :, R], 1.0)
            gaug = [persist.tile([R + 1, 512], F32R, name=f"gaug{j}")
                    for j in range(2)]
            for j in range(4):
                nc.vector.memset(tTb[j].bitcast(F32)[R : R + 1, :], 1.0)
            for j in range(2):
                nc.vector.memset(gaug[j].bitcast(F32)[R : R + 1, :], 1.0)

            with (
                tc.tile_pool(name="xin", bufs=4) as xin_pool,
                tc.tile_pool(name="xtb", bufs=2) as xtb_pool,
                tc.tile_pool(name="exp", bufs=6) as exp_pool,
                tc.tile_pool(name="fin_sb", bufs=2) as fin_sb,
                tc.tile_pool(name="ysb", bufs=4) as ysb_pool,
                tc.tile_pool(name="ps_main", bufs=2, space="PSUM") as ps_main,
                tc.tile_pool(name="ps_uz", bufs=2, space="PSUM") as ps_uz,
                tc.tile_pool(name="ps_g", bufs=1, space="PSUM") as ps_g,
            ):
                # ---- k/q projection pieces --------------------------------
                # Prep-phase pieces ride the plentiful early "sc" slots;
                # steady-state pieces use the spare 1-bank "cty" tag so they
                # never perturb the exp pipeline's sc rotation.
                def k_piece(h, blk, tag="cty"):   # K cols [512*blk, +512)
                    pp = ps_main.tile([R, 512], F32, tag=tag,
                                      bufs=2 if tag == "sc" else 1,
                                      name=f"kp_{h}_{blk}")
                    nc.tensor.matmul(
                        pp,
                        lhsT=wk_sb[:, R * h : R * (h + 1)],
                        rhs=tTb[blk],
                    )
                    nc.vector.tensor_copy(
                        K_sb[:, h, 512 * blk : 512 * (blk + 1)], pp
                    )

                def q_piece(h, sbq, tag="cty"):   # Q cols [512*sbq, +512)
                    pp = ps_main.tile([R, 512], F32, tag=tag,
                                      bufs=2 if tag == "sc" else 1,
                                      name=f"qp_{h}_{sbq}")
                    nc.tensor.matmul(
                        pp,
                        lhsT=wq_sb[:, R * h : R * (h + 1)],
                        rhs=tTb[sbq],
                    )
                    nc.vector.tensor_copy(
                        Q_sb[:, h, 512 * sbq : 512 * (sbq + 1)], pp
                    )

                # ---- prep: transpose x on the PE (bf16, 1 cyc/row) -----
                def x_block(blk):         # 256 x-rows [256*blk, 256*blk+256)
                    x_q = xin_pool.tile([128, 2, D], BF16, tag="xq",
                                        name=f"xq{blk}")
                    nc.sync.dma_start(
                        out=x_q,
                        in_=xb.rearrange("(a p) d -> p a d", p=128)[
                            :, 2 * blk : 2 * blk + 2, :
                        ],
                    )
                    xT_blk = xtb_pool.tile([128, 2, 8, 128], BF16, tag="xT")
                    for sc2 in range(2):
                        tr = ps_main.tile([128, 8, 128], BF16, tag="sc",
                                          bufs=2, name=f"tr{blk}_{sc2}")
                        for dc in range(8):
                            nc.tensor.matmul(
                                tr[:, dc, :],
                                lhsT=x_q[:, sc2, 128 * dc : 128 * (dc + 1)],
                                rhs=identb,
                                is_transpose=True,
                                start=(dc == 0),
                                stop=(dc == 7),
                            )
                        nc.vector.tensor_copy(xT_blk[:, sc2, :, :], tr)
                    tt_ps = ps_main.tile([R, 256], F32, tag="sc", bufs=2,
                                         name=f"tt{blk}")
                    for dc in range(8):
                        nc.tensor.matmul(
                            tt_ps,
                            lhsT=qkvu_sb[:, dc, :],
                            rhs=xT_blk[:, :, dc, :],
                            start=(dc == 0),
                            stop=(dc == 7),
                        )
                    nc.vector.tensor_copy(
                        tTb[blk // 2][0:R, 256 * (blk % 2) : 256 * (blk % 2) + 256],
                        tt_ps,
                    )

                def vls_cty(tcc):
                    vl = ps_main.tile([128, 512], F32, tag="cty", bufs=1,
                                      name=f"vlc{tcc}")
                    nc.tensor.matmul(
                        vl,
                        lhsT=tTb[tcc // 4][:, 128 * (tcc % 4) : 128 * (tcc % 4) + 128],
                        rhs=wv_sb,
                    )
                    nc.vector.tensor_copy(
                        V_sb[:, tcc, :, 0:R],
                        vl.rearrange("p (h r) -> p h r", h=H),
                    )

                def vls(tcc0, n):
                    for tcc in range(tcc0, tcc0 + n):
                        vl = ps_main.tile([128, 512], F32, tag="sc",
                                          bufs=2, name=f"vl{tcc}")
                        nc.tensor.matmul(
                            vl,
                            lhsT=tTb[tcc // 4][:, 128 * (tcc % 4) : 128 * (tcc % 4) + 128],
                            rhs=wv_sb,
                        )
                        nc.vector.tensor_copy(
                            V_sb[:, tcc, :, 0:R],
                            vl.rearrange("p (h r) -> p h r", h=H),
                        )

                # ---- attention: flat software-pipelined tile stream ----
                # 256 tiles t: sbq = t//128, h = (t%128)//8, tp = t%8.
                # Iteration t emits [exp(t)] [AV(t-1)] [scores(<=t+2)]
                # [deferred fin stages] [one k/q or y piece].  AV precedes
                # the scores emission so the PE never parks a ready AV
                # behind a scores matmul that waits on its PSUM bank; fin
                # stages are emitted 2/4 tiles after their data deps start
                # so cross-engine chains never park an engine queue.
                pending = []

                def run_piece():
                    if pending:
                        pending.pop(0)()

                NT = 256

                def tile_idx(t):
                    return t // 128, (t % 128) // 8, t % 8

                sc_tiles = {}
                ex_tiles = {}
                uz_tiles = {}

                def emit_scores(t):
                    sbq, h, tp = tile_idx(t)
                    sc = ps_main.tile([128, 1024], F32, tag="sc",
                                      name=f"sc_{t}")
                    sc_tiles[t] = sc
                    for i in range(2):
                        tcc = 2 * tp + i
                        nc.tensor.matmul(
                            sc[:, 512 * i : 512 * (i + 1)],
                            lhsT=K_sb[:, h, 128 * tcc : 128 * (tcc + 1)],
                            rhs=Q_sb[:, h, 512 * sbq : 512 * (sbq + 1)],
                        )

                def emit_exp(t):
                    ex = exp_pool.tile([128, 1024], BF16, tag="ex",
                                       name=f"ex_{t}")
                    ex_tiles[t] = ex
                    nc.scalar.activation(ex, sc_tiles.pop(t), EXP,
                                         bias=zeros_col)

                def emit_av(t):
                    sbq, h, tp = tile_idx(t)
                    hh = 16 * sbq + h
                    if tp == 0:
                        uz_tiles[hh] = ps_uz.tile(
                            [R + 1, 512], F32, tag="uz", name=f"uz_{hh}"
                        )
                    uzs = uz_tiles[hh]
                    ex = ex_tiles.pop(t)
                    for i in range(2):
                        tcc = 2 * tp + i
                        nc.tensor.matmul(
                            uzs,
                            lhsT=V_sb[:, tcc, h, :],
                            rhs=ex[:, 512 * i : 512 * (i + 1)],
                            start=(tp == 0 and i == 0),
                            stop=(tp == 7 and i == 1),
                        )

                def fin_a(hh):            # hh = global head index 0..31
                    uzs = uz_tiles[hh]
                    zr = fin_sb.tile([1, 512], F32R, tag="zr",
                                     name=f"zr_{hh}")
                    with nc.allow_low_precision(reason="fp32r attn"):
                        nc.vector.reciprocal(zr, uzs[R : R + 1, :])
                    zr32 = fin_sb.tile([R, 512], F32R, tag="zr32",
                                       name=f"zr32_{hh}")
                    nc.gpsimd.partition_broadcast(zr32, zr[0:1, :])
                    return zr32

                def fin_b(hh, zr32):
                    uzs = uz_tiles.pop(hh)
                    ulow = fin_sb.tile([R, 512], F32R, tag="ulow",
                                       name=f"ul_{hh}")
                    nc.vector.tensor_mul(ulow, uzs[0:R, :], zr32)
                    return ulow

                def fin_g(hh, ulow):
                    sbq, h = hh // 16, hh % 16
                    if h == 0:
                        g_ps[sbq] = ps_g.tile([R, 512], F32, tag="g",
                                              name=f"g{sbq}")
                    nc.tensor.matmul(
                        g_ps[sbq],
                        lhsT=ident[0:R, 0:R],
                        rhs=ulow,
                        start=(h == 0),
                        stop=(h == H - 1),
                    )

                g_ps = {}

                def head_pieces(hh):      # pieces feeding global head hh
                    sbq, h = hh // 16, hh % 16
                    if sbq == 0:
                        return [lambda b=b: k_piece(h, b) for b in range(4)] \
                             + [lambda: q_piece(h, 0)]
                    return [lambda: q_piece(h, 1)]

                def y_pieces(sbq):
                    def half(yc, nb):
                        def run():
                            ga = gaug[sbq]
                            y_ps = ps_main.tile([128, 512], F32, tag="cty",
                                                bufs=1, name=f"y_{sbq}_{yc}_{nb}")
                            nc.tensor.matmul(
                                y_ps,
                                lhsT=ga[:, 128 * yc : 128 * (yc + 1)],
                                rhs=outv_sb[:, 512 * nb : 512 * (nb + 1)],
                            )
                            y_h = ysb_pool.tile([128, 512], F32, tag="ysb",
                                                name=f"yh_{sbq}_{yc}_{nb}")
                            nc.vector.tensor_copy(y_h, y_ps)
                            row0 = 512 * sbq + 128 * yc
                            nc.sync.dma_start(
                                out=y[row0 : row0 + 128,
                                      512 * nb : 512 * (nb + 1)],
                                in_=y_h,
                            )
                        return run

                    return [half(yc, nb) for yc in range(4) for nb in range(2)]

                ysb = {}
                actions = {}

                def at(t, thunk):
                    actions.setdefault(t, []).append(thunk)

                # steady-state k/q pieces: appended three head-slots
                # ahead of need, drained one per iteration
                append_at = {8 * (hh - 2) + 1: [hh] for hh in range(2, 32)}

                # Prologue.  s-half-0 transpose chunks first: tT blocks
                # 0/1, head 0/1 low-key pieces and scores tiles 0..7 are all
                # live ~time the s-half-1 chunks land.  vls 4-15 trail in
                # the sc FIFO behind the prologue scores; the 10-deep ex
                # pool keeps the first exps slot-free until the loop's AV
                # emissions catch up (they sit behind the vl matmuls in the
                # PE queue).
                x_block(0)
                nc.sync.dma_start(out=wv_sb, in_=wv)
                nc.sync.dma_start(out=wk_sb, in_=wk)
                nc.sync.dma_start(out=wq_sb, in_=wq)
                x_block(1)
                x_block(2)
                x_block(3)
                x_block(4)
                x_block(5)
                x_block(6)
                x_block(7)
                nc.sync.dma_start(out=outv_sb, in_=outv)
                k_piece(0, 0, tag="sc")
                q_piece(0, 0, tag="sc")
                k_piece(0, 1, tag="sc")
                vls(0, 4)
                k_piece(1, 0)
                k_piece(1, 1)
                q_piece(1, 0)
                k_piece(0, 2, tag="sc")
                k_piece(0, 3, tag="sc")
                k_piece(1, 2)
                k_piece(1, 3)
                for t0 in range(8):
                    emit_scores(t0)
                sc_next = 8
                vls(4, 4)
                # vls 8-15 drip through the pending queue (cty tag) so
                # their sc slots never stall the loop's scores tiles
                pending.extend(
                    lambda tcc=tcc: vls_cty(tcc) for tcc in range(8, 16)
                )

                fin_state = {}
                for t in range(NT + 6):
                    if t < NT:
                        emit_exp(t)
                    if 1 <= t <= NT:
                        emit_av(t - 1)
                        sbq, h, tp = tile_idx(t - 1)
                        hh = 16 * sbq + h
                        if tp == 7:
                            last = hh == 31
                            at(t + 1, lambda hh=hh: fin_state.__setitem__(
                                hh, fin_a(hh)))
                            at(t + (2 if last else 3),
                               lambda hh=hh: fin_state.__setitem__(
                                   hh, fin_b(hh, fin_state.pop(hh))))
                            at(t + (3 if last else 5),
                               lambda hh=hh: fin_g(hh, fin_state.pop(hh)))
                            if hh == 15:
                                def g0_out():
                                    nc.vector.tensor_copy(
                                        gaug[0][0:R, :], g_ps[0]
                                    )
                                at(t + 6, g0_out)
                                at(t + 6, lambda: pending.extend(y_pieces(0)))
                    for hh2 in append_at.pop(t, []):
                        pending.extend(head_pieces(hh2))
                    while sc_next < min(t + 3, NT):
                        emit_scores(sc_next)
                        sc_next += 1
                    for thunk in actions.pop(t, []):
                        thunk()
                    run_piece()
                while pending:
                    run_piece()

                # half 1's output projection: pipeline is drained, borrow
                # the sc slots; copies alternate DVE/ACT (both idle) and
                # every 512-half DMAs out as soon as it is copied
                nc.vector.tensor_copy(gaug[1][0:R, :], g_ps[1])
                for yc in range(4):
                    y_ps = ps_main.tile([128, 1024], F32, tag="sc",
                                        name=f"y_1_{yc}")
                    row0 = 512 + 128 * yc
                    for nb in range(2):
                        nc.tensor.matmul(
                            y_ps[:, 512 * nb : 512 * (nb + 1)],
                            lhsT=gaug[1][:, 128 * yc : 128 * (yc + 1)],
                            rhs=outv_sb[:, 512 * nb : 512 * (nb + 1)],
                        )
                        y_h = ysb_pool.tile([128, 512], F32, tag="ysb",
                                            name=f"yh_1_{yc}_{nb}")
                        ysrc = y_ps[:, 512 * nb : 512 * (nb + 1)]
                        if nb == 0:
                            nc.vector.tensor_copy(y_h, ysrc)
                        else:
                            nc.scalar.copy(y_h, ysrc)
                        nc.sync.dma_start(
                            out=y[row0 : row0 + 128,
                                  512 * nb : 512 * (nb + 1)],
                            in_=y_h,
                        )

    nc.compile()
    return nc


def _host_params(qkv_u, qkv_v, qkv_b, u_attn, v_attn, out_u, out_v, out_b):
    scale = np.float32(1.0 / np.sqrt(np.float32(R)))
    Vq, Vk, Vv = qkv_v[:, :D], qkv_v[:, D : 2 * D], qkv_v[:, 2 * D :]
    bq_f, bk_f, bv_f = qkv_b[:D], qkv_b[D : 2 * D], qkv_b[2 * D :]

    wq = np.zeros((R + 1, H * R), np.float32)
    wk = np.zeros((R + 1, H * R), np.float32)
    wv = np.zeros((R + 1, H * R), np.float32)
    for h in range(H):
        U = u_attn[h]  # [HD, R]
        sl = slice(R * h, R * (h + 1))
        hd = slice(HD * h, HD * (h + 1))
        # W folds the per-head output projection into the AV weights:
        # uz rows land directly in g-space (head-summed by an identity
        # accumulation matmul).
        W = v_attn[h] @ out_u[hd, :]  # [R, R]
        wq[:R, sl] = (Vq[:, hd] @ U) * scale
        wq[R, sl] = (bq_f[hd] @ U) * scale
        wk[:R, sl] = Vk[:, hd] @ U
        wk[R, sl] = bk_f[hd] @ U
        wv[:R, sl] = (Vv[:, hd] @ U) @ W
        wv[R, sl] = (bv_f[hd] @ U) @ W

    outv_aug = np.concatenate([out_v, out_b[None, :]], axis=0).astype(np.float32)

    import ml_dtypes
    return dict(
        wq=wq, wk=wk, wv=wv,
        qkvu=np.ascontiguousarray(qkv_u.astype(ml_dtypes.bfloat16)),
        outv=outv_aug,
        ones2048=np.ones((1, S), np.float32),
    )


_NC_CACHE = None
LAST_RESULTS = None


def kernel(x, mask, qkv_u, qkv_v, qkv_b, u_attn, v_attn, out_u, out_v, out_b):
    global _NC_CACHE, LAST_RESULTS
    x = np.asarray(x, dtype=np.float32)
    params = _host_params(
        np.asarray(qkv_u, np.float32), np.asarray(qkv_v, np.float32),
        np.asarray(qkv_b, np.float32), np.asarray(u_attn, np.float32),
        np.asarray(v_attn, np.float32), np.asarray(out_u, np.float32),
        np.asarray(out_v, np.float32), np.asarray(out_b, np.float32),
    )
    # mask is all-ones by construction (spec fill=ones): masking is a no-op.

    if _NC_CACHE is None:
        _NC_CACHE = build_program()
    nc = _NC_CACHE

    import ml_dtypes
    xb16 = x.astype(ml_dtypes.bfloat16)
    in_maps = []
    for c in range(NC):
        b, sh = c // 2, c % 2
        if sh == 0:
            xc = xb16[b]
        else:
            xc = np.concatenate([xb16[b, SHALF:], xb16[b, :SHALF]], axis=0)
        in_maps.append(dict(params, xb=np.ascontiguousarray(xc)))

    trace = os.environ.get("KERNEL_TRACE", "0") == "1"
    res = run_bass_kernel_spmd(nc, in_maps, list(range(NC)), trace=trace)
    LAST_RESULTS = res

    out = np.empty((B, S, D), np.float32)
    for c in range(NC):
        b, sh = c // 2, c % 2
        out[b, SHALF * sh : SHALF * (sh + 1)] = res.results[c]["y"]
    return out
